# revision 12
# baseline (speedup 1.0000x reference)
"""Sparse (block-local) attention for B=2, Sq=2048, Sk=4096, D=1024, H=16.

Each query i attends to exactly keys {2i, 2i+1} (Sk/Sq == 2, no remainder),
so softmax is over 2 scores -> p1 = sigmoid((s1-s2)*scale), p2 = 1-p1.

Distribution: sequence-parallel over (batch, query-block). 8 cores, each takes
512 contiguous queries of one batch plus the matching 1024 contiguous keys.
No collectives needed; outputs are concatenated on the host.

Algebraic cuts: with exactly 2 keys per query, softmax only needs the score
DIFFERENCE, and k_even - k_odd = (c_even - c_odd) @ Wk^T is linear, so the K
projection runs on c_diff = c_even - c_odd (512 rows, not 1024). Likewise
att = v_odd + p1 * (v_even - v_odd) reuses c_diff for V, and the v_odd term
folds through the output projection with a host-precomputed weight product
Wvo = Wo @ Wv:
  out = c_odd @ Wvo^T + (p1 * Vd) @ Wo^T,  Vd = c_diff @ Wv^T

Per-core device kernel (fp32 PSUM accumulation everywhere):
  Q  = x_s @ Wq^T     fp8 e4m3 DoubleRow (2 contraction rows/cycle, 2x rate)
  Kd = c_diff @ Wk^T  fp8 e4m3 DoubleRow
  s-diff row-wise dots on DVE per 64-dim head; p1 on ACT (sigmoid)
  Vd = c_diff @ Wv^T  bf16 (feeds the output directly -> fp8 too lossy)
  av = p1 * Vd on DVE; av^T via PE transposes
  O  = c_odd @ Wvo^T + av^T-matmul @ Wo^T   bf16

fp8 error budget (verified against an exact numpy replica): Wq/Wk ship as
e4m3 pre-scaled by 32 (folded into the sigmoid scale), x and c_diff as plain
e4m3. Only the sigmoid INPUT sees the quantization noise, so the output rel
err is ~1.6e-2 vs the 2e-2 gate; bf16 everywhere the error hits the output
linearly.

DMA: fine-grained whole tensors in phase need-order split across both hwdge
rings, no completion chains (per-ring FIFO keeps order); phase order is
pinned via tile_wait_until. Output copies split ACT/DVE into 256-col halves
DMA'd on both rings to shorten the tail.
"""

import sys

for _p in ("/opt/trn_rl_repo",):
    if _p not in sys.path:
        sys.path.append(_p)

import numpy as np
import ml_dtypes

import concourse.bass as bass
import concourse.mybir as mybir
import concourse.tile as tile
from concourse import bacc
from concourse.bass_utils import run_bass_kernel_spmd
from concourse.masks import make_identity
from concourse.tile_rust import add_dep_helper

B, SQ, SK, D, H, HD = 2, 2048, 4096, 1024, 16, 64
N_CORES = 8
QL = B * SQ // N_CORES       # 512 queries per core
KL = 2 * QL                  # 1024 keys per core
QT = QL // 128               # 4 query tiles
NB = 512                     # psum bank width (fp32)
JT = D // NB                 # 2 output-column blocks per projection
DT = D // 128                # 8 feature tiles
SCALE = 1.0 / float(np.sqrt(HD))

FB = mybir.dt.bfloat16
F32 = mybir.dt.float32
F8 = mybir.dt.float8e4
BF = ml_dtypes.bfloat16
E4M3 = ml_dtypes.float8_e4m3fn
WSCALE = 32.0


def _build(kd_tiles: int, with_bo: bool, fp8: bool = False):
    """Build + finalize the per-core Bacc graph (SPMD: same graph on 8 cores).

    fp8=True is the fast path for the bias-free case; the general
    (with-bias) path keeps everything bf16 with bias rows augmented into
    the contraction dim.
    """
    if fp8:
        return _build_fp8()
    nc = bacc.Bacc("TRN2", target_bir_lowering=False)

    # All activation/weight inputs are host-arranged partition-major:
    # tensor[p, t, n] = logical[t*128 + p, n], so DMA descriptors are
    # per-partition contiguous. Inputs are merged by NEED ORDER and the
    # DMA chain is gated so each phase gets full HBM bandwidth.
    X0Q = 3 * 128               # x columns (queries) in xw0
    xw0 = nc.dram_tensor("xw0", [128, kd_tiles, X0Q + NB], FB,
                         kind="ExternalInput")
    xw1 = nc.dram_tensor("xw1", [128, kd_tiles, (QL - X0Q) + (D - NB)], FB,
                         kind="ExternalInput")
    ck = nc.dram_tensor("ck", [128, kd_tiles, QL + D], FB, kind="ExternalInput")
    cv = nc.dram_tensor("cv", [128, kd_tiles, QL + D], FB, kind="ExternalInput")
    woo = nc.dram_tensor("woo", [128, kd_tiles, 2 * D], FB,
                         kind="ExternalInput")
    bo = None
    if with_bo:
        bo = nc.dram_tensor("bo", [1, D], F32, kind="ExternalInput")
    out = nc.dram_tensor("out", [128, QT, D], F32, kind="ExternalOutput")

    with tile.TileContext(nc) as tc:
        with (
            tc.tile_pool(name="ins", bufs=1) as ins,
            tc.tile_pool(name="acts", bufs=1) as acts,
            tc.tile_pool(name="att", bufs=4) as att,
            tc.tile_pool(name="outs", bufs=4) as outs,
            tc.tile_pool(name="psum", bufs=6, space="PSUM") as psum,
            tc.tile_pool(name="psum_tr", bufs=2, space="PSUM") as psum_tr,
        ):
            # ---- inputs to SBUF (need-order chained DMAs) ------------------
            xw0_sb = ins.tile([128, kd_tiles, X0Q + NB], FB)
            xw1_sb = ins.tile([128, kd_tiles, (QL - X0Q) + (D - NB)], FB)
            ck_sb = ins.tile([128, kd_tiles, QL + D], FB)
            cv_sb = ins.tile([128, kd_tiles, QL + D], FB)
            woo_sb = ins.tile([128, kd_tiles, 2 * D], FB)
            ident = ins.tile([128, 128], FB)

            h0 = (X0Q + NB) // 2
            d0a = nc.sync.dma_start(out=xw0_sb[:, :, 0:h0], in_=xw0[:, :, 0:h0])
            d0b = nc.scalar.dma_start(out=xw0_sb[:, :, h0:], in_=xw0[:, :, h0:])
            d1 = nc.sync.dma_start(out=xw1_sb, in_=xw1[:])
            d2 = nc.sync.dma_start(out=ck_sb, in_=ck[:])
            d3 = nc.sync.dma_start(out=cv_sb, in_=cv[:])
            d4 = nc.sync.dma_start(out=woo_sb, in_=woo[:])
            for d0x in (d0a, d0b):
                add_dep_helper(d1.ins, d0x.ins, sync=True)
                add_dep_helper(d2.ins, d0x.ins, sync=True)
            add_dep_helper(d3.ins, d1.ins, sync=True)
            add_dep_helper(d3.ins, d2.ins, sync=True)
            add_dep_helper(d4.ins, d3.ins, sync=True)
            bo_sb = None
            if with_bo:
                bo_sb = ins.tile([128, D], F32)
                d5 = nc.sync.dma_start(out=bo_sb,
                                       in_=bo[:].to_broadcast((128, D)))
                add_dep_helper(d5.ins, d3.ins, sync=True)
            make_identity(nc, ident)

            # PE warm-up: dummy matmuls during the DMA head keep HAM busy so
            # the real stream starts at full clock, at zero wall-clock cost.
            warm = ins.tile([128, 128], FB)
            nc.vector.memset(warm, 1.0)
            wps = psum_tr.tile([128, 128], F32, tag="tr")
            for _ in range(110):
                nc.tensor.matmul(wps, lhsT=warm, rhs=warm, start=True, stop=True)

            def x_slice(kd, col0):
                if col0 < X0Q:
                    return xw0_sb[:, kd, col0:col0 + 128]
                c = col0 - X0Q
                return xw1_sb[:, kd, c:c + 128]

            def wq_slice(kd, jb):
                if jb == 0:
                    return xw0_sb[:, kd, X0Q:X0Q + NB]
                c = (QL - X0Q) + (jb - 1) * NB
                return xw1_sb[:, kd, c:c + NB]

            def cdiff_slice(kd, col0):
                return ck_sb[:, kd, col0:col0 + 128]

            def wk_slice(kd, jb):
                return ck_sb[:, kd, QL + jb * NB:QL + (jb + 1) * NB]

            def codd_slice(kd, col0):
                return cv_sb[:, kd, col0:col0 + 128]

            def wv_slice(kd, jb):
                return cv_sb[:, kd, QL + jb * NB:QL + (jb + 1) * NB]

            # ---- projections (psum copies all on ACT) ----------------------
            q_sb = acts.tile([128, QT, D], FB)           # Q row-major
            kd_sb = acts.tile([128, QT, D], FB)          # Kd = c_diff @ Wk^T
            v_sb = acts.tile([128, QT, D], FB)           # Vd = c_diff @ Wv^T

            def mm_one(dst_tile, dst_idx, jb, lhs_fn, rhs_fn, nkd=kd_tiles):
                ps = psum.tile([128, NB], F32, tag="mm")
                for kd in range(nkd):
                    nc.tensor.matmul(
                        ps,
                        lhsT=lhs_fn(kd),
                        rhs=rhs_fn(kd, jb),
                        start=(kd == 0),
                        stop=(kd == nkd - 1),
                    )
                nc.scalar.copy(dst_tile[:, dst_idx, jb * NB:(jb + 1) * NB], ps)

            def mm_group(dst_tile, dst_idx, lhs_fn, rhs_fn):
                for jb in range(JT):
                    mm_one(dst_tile, dst_idx, jb, lhs_fn, rhs_fn)

            av_sb = acts.tile([128, QT, D], FB)

            def attention(qt):
                qv = q_sb[:, qt, :]
                kdv = kd_sb[:, qt, :]
                pe = att.tile([128, H, HD], FB, tag="prod")
                nc.vector.tensor_mul(pe.rearrange("p h e -> p (h e)"), qv, kdv)
                ds = att.tile([128, H], F32, tag="s")
                nc.vector.reduce_sum(out=ds, in_=pe, axis=mybir.AxisListType.X)
                p1 = att.tile([128, H], F32, tag="s")
                nc.scalar.activation(p1, ds, mybir.ActivationFunctionType.Sigmoid,
                                     scale=SCALE)
                vd = v_sb[:, qt, :].rearrange("p (h e) -> p h e", h=H)
                nc.vector.tensor_mul(
                    av_sb[:, qt, :].rearrange("p (h e) -> p h e", h=H),
                    vd, p1.to_broadcast((128, H, HD)))

            for jb in range(JT):
                for qt in range(QT):
                    mm_one(q_sb, qt, jb,
                           lambda kd, qt=qt: x_slice(kd, qt * 128), wq_slice)
            for qt in range(QT):
                mm_group(kd_sb, qt,
                         lambda kd, qt=qt: cdiff_slice(kd, qt * 128), wk_slice)
            for qt in range(QT):
                mm_group(v_sb, qt,
                         lambda kd, qt=qt: cdiff_slice(kd, qt * 128), wv_slice)
                if qt >= 1:
                    attention(qt - 1)
            attention(QT - 1)

            # ---- transpose att -> attT (copies on ACT), O groups interleaved
            avT_sb = acts.tile([128, DT, QL], FB)        # att^T feature-major

            def transposes(qt):
                for db in range(DT):
                    tp = psum_tr.tile([128, 128], FB, tag="tr")
                    nc.tensor.transpose(tp, av_sb[:, qt, db * 128:(db + 1) * 128],
                                        ident)
                    nc.scalar.copy(avT_sb[:, db, qt * 128:(qt + 1) * 128], tp)

            def o_group(qt):
                pss = [psum.tile([128, NB], F32, tag="mm", name=f"psg{jb}") for jb in range(JT)]
                for jb in range(JT):
                    for kd in range(kd_tiles):
                        nc.tensor.matmul(
                            pss[jb],
                            lhsT=codd_slice(kd, qt * 128),
                            rhs=woo_sb[:, kd, D + jb * NB:D + (jb + 1) * NB],
                            start=(kd == 0),
                            stop=False,
                        )
                    for kd in range(DT):
                        nc.tensor.matmul(
                            pss[jb],
                            lhsT=avT_sb[:, kd, qt * 128:(qt + 1) * 128],
                            rhs=woo_sb[:, kd, jb * NB:(jb + 1) * NB],
                            start=False,
                            stop=(kd == DT - 1),
                        )
                for jb in range(JT):
                    o_t = outs.tile([128, NB], F32, tag="o")
                    if with_bo:
                        nc.vector.tensor_add(o_t, pss[jb],
                                             bo_sb[:, jb * NB:(jb + 1) * NB])
                    elif jb % 2 == 0:
                        nc.scalar.copy(o_t, pss[jb])
                    else:
                        nc.vector.tensor_copy(o_t, pss[jb])
                    nc.sync.dma_start(out=out[:, qt, jb * NB:(jb + 1) * NB],
                                      in_=o_t)

            transposes(0)
            transposes(1)
            o_group(0)
            transposes(2)
            o_group(1)
            transposes(3)
            o_group(2)
            o_group(3)

    nc.finalize()
    return nc


def _build_fp8():
    """Bias-free fast path: fp8 DoubleRow Q/Kd/Vd (centered form), bf16 O.

    Centered attention: att = v_mean + (p1 - 1/2) * vd  with
    v_mean = (v_even + v_odd)/2, so the Vd term's coefficient
    t/2 = (p1 - 1/2) = tanh(z/2)/2 has RMS ~0.15 instead of ~0.5 —
    attenuating Vd's fp8 quantization error ~3.6x and bringing fp8 Vd
    inside the error budget. Scales: Wv ships as e4m3(32*Wv) so
    v_sb = 64*(c_diff @ (Wv/2)^T); the 1/64 is folded into wo = Wo/64.
    """
    nc = bacc.Bacc("TRN2", target_bir_lowering=False)
    kd_tiles = DT
    DR = mybir.MatmulPerfMode.DoubleRow

    xq0 = nc.dram_tensor("xq0", [128, kd_tiles, 128], F8, kind="ExternalInput")
    xq1 = nc.dram_tensor("xq1", [128, kd_tiles, 128], F8, kind="ExternalInput")
    xq2 = nc.dram_tensor("xq2", [128, kd_tiles, 128], F8, kind="ExternalInput")
    xq3 = nc.dram_tensor("xq3", [128, kd_tiles, 128], F8, kind="ExternalInput")
    wq0a = nc.dram_tensor("wq0a", [128, kd_tiles, NB // 2], F8,
                          kind="ExternalInput")
    wq0b = nc.dram_tensor("wq0b", [128, kd_tiles, NB // 2], F8,
                          kind="ExternalInput")
    wq1 = nc.dram_tensor("wq1", [128, kd_tiles, NB], F8, kind="ExternalInput")
    cdf8 = nc.dram_tensor("cdf8", [128, kd_tiles, QL], F8,
                          kind="ExternalInput")
    wk0 = nc.dram_tensor("wk0", [128, kd_tiles, NB], F8, kind="ExternalInput")
    wk1 = nc.dram_tensor("wk1", [128, kd_tiles, NB], F8, kind="ExternalInput")
    wv0 = nc.dram_tensor("wv0", [128, kd_tiles, NB], F8, kind="ExternalInput")
    wv1 = nc.dram_tensor("wv1", [128, kd_tiles, NB], F8, kind="ExternalInput")
    cod = nc.dram_tensor("cod", [128, kd_tiles, QL], FB, kind="ExternalInput")
    wvo = nc.dram_tensor("wvo", [128, kd_tiles, D], FB, kind="ExternalInput")
    wo = nc.dram_tensor("wo", [128, kd_tiles, D], FB, kind="ExternalInput")
    out = nc.dram_tensor("out", [128, QT, D], F32, kind="ExternalOutput")

    with tile.TileContext(nc) as tc:
        with (
            tc.tile_pool(name="ins", bufs=1) as ins,
            tc.tile_pool(name="acts", bufs=1) as acts,
            tc.tile_pool(name="att", bufs=4) as att,
            tc.tile_pool(name="outs", bufs=8) as outs,
            tc.tile_pool(name="psum", bufs=5, space="PSUM") as psum,
            tc.tile_pool(name="psum_tr", bufs=2, space="PSUM") as psum_tr,
            tc.tile_pool(name="psum_w", bufs=1, space="PSUM") as psum_w,
        ):
            xq_sb = [ins.tile([128, kd_tiles, 128], F8, name=f"xq{i}")
                     for i in range(QT)]
            wq0a_sb = ins.tile([128, kd_tiles, NB // 2], F8)
            wq0b_sb = ins.tile([128, kd_tiles, NB // 2], F8)
            wq1_sb = ins.tile([128, kd_tiles, NB], F8)
            cdf8_sb = ins.tile([128, kd_tiles, QL], F8)
            wk0_sb = ins.tile([128, kd_tiles, NB], F8)
            wk1_sb = ins.tile([128, kd_tiles, NB], F8)
            wv0_sb = ins.tile([128, kd_tiles, NB], F8)
            wv1_sb = ins.tile([128, kd_tiles, NB], F8)
            cod_sb = ins.tile([128, kd_tiles, QL], FB)
            wvo_sb = ins.tile([128, kd_tiles, D], FB)
            wo_sb = ins.tile([128, kd_tiles, D], FB)
            ident = ins.tile([128, 128], FB)

            # unchained: per-ring FIFO keeps need-order. Q-critical data is
            # split across both rings' crawl windows; each later phase's
            # tensors are balanced so both rings deliver it about when the
            # PE stream reaches it.
            nc.sync.dma_start(out=wq0a_sb, in_=wq0a[:])
            nc.sync.dma_start(out=xq_sb[1], in_=xq1[:])
            nc.sync.dma_start(out=xq_sb[2], in_=xq2[:])
            nc.sync.dma_start(out=xq_sb[3], in_=xq3[:])
            nc.sync.dma_start(out=wk0_sb, in_=wk0[:])
            nc.sync.dma_start(out=wv0_sb, in_=wv0[:])
            nc.sync.dma_start(out=cod_sb, in_=cod[:])
            nc.sync.dma_start(out=wo_sb, in_=wo[:])
            nc.scalar.dma_start(out=xq_sb[0], in_=xq0[:])
            nc.scalar.dma_start(out=wq0b_sb, in_=wq0b[:])
            nc.scalar.dma_start(out=wq1_sb, in_=wq1[:])
            nc.scalar.dma_start(out=cdf8_sb, in_=cdf8[:])
            nc.scalar.dma_start(out=wk1_sb, in_=wk1[:])
            nc.scalar.dma_start(out=wv1_sb, in_=wv1[:])
            nc.scalar.dma_start(out=wvo_sb, in_=wvo[:])
            make_identity(nc, ident)

            # PE warm-up holds the p-state ramp until the first Q data lands
            warm = ins.tile([128, 128], FB)
            nc.vector.memset(warm, 1.0)
            wps = psum_w.tile([128, 128], F32, tag="warm")
            for _ in range(44):
                nc.tensor.matmul(wps, lhsT=warm, rhs=warm, start=True,
                                 stop=True)

            def x2(t, qt):
                # fp8 DoubleRow lhsT: contraction pair (2t, 2t+1), 128 q cols
                return xq_sb[qt][:, 2 * t:2 * t + 2, :]

            def cdiff8_2(t, qt):
                return cdf8_sb[:, 2 * t:2 * t + 2, qt * 128:(qt + 1) * 128]

            def codd_slice(kd, col0):
                return cod_sb[:, kd, col0:col0 + 128]

            q_sb = acts.tile([128, QT, D], FB)
            kd_sb = acts.tile([128, QT, D], FB)
            v_sb = acts.tile([128, QT, D], FB)

            def mm_dr(dst_tile, dst_idx, jb, lhs_fn, rhs_sb):
                # 4 DoubleRow matmuls, 256-contraction each
                ps = psum.tile([128, NB], F32, tag="mm")
                for t in range(kd_tiles // 2):
                    nc.tensor.matmul(
                        ps,
                        lhsT=lhs_fn(t),
                        rhs=rhs_sb[:, 2 * t:2 * t + 2, :],
                        start=(t == 0),
                        stop=(t == kd_tiles // 2 - 1),
                        perf_mode=DR,
                    )
                nc.scalar.copy(dst_tile[:, dst_idx, jb * NB:(jb + 1) * NB], ps)

            av_sb = acts.tile([128, QT, D], FB)

            # t = tanh(z/2) = 2*(p1 - 1/2); av = t * (64 * c_diff@(Wv/2)^T),
            # the 1/64 folded into wo host-side
            t_tiles = [None] * QT

            def score(qt):
                qv = q_sb[:, qt, :]
                kdv = kd_sb[:, qt, :]
                pe = att.tile([128, H, HD], FB, tag="prod")
                nc.vector.tensor_mul(pe.rearrange("p h e -> p (h e)"), qv, kdv)
                ds = att.tile([128, H], F32, tag="s")
                nc.vector.reduce_sum(out=ds, in_=pe, axis=mybir.AxisListType.X)
                t = att.tile([128, H], F32, tag="p1")
                nc.scalar.activation(t, ds,
                                     mybir.ActivationFunctionType.Tanh,
                                     scale=SCALE / (2 * WSCALE * WSCALE))
                t_tiles[qt] = t

            def avmul(qt):
                vd = v_sb[:, qt, :].rearrange("p (h e) -> p h e", h=H)
                nc.vector.tensor_mul(
                    av_sb[:, qt, :].rearrange("p (h e) -> p h e", h=H),
                    vd, t_tiles[qt].to_broadcast((128, H, HD)))

            # Q jb0 in 256-col halves so the first groups gate on the
            # smallest possible front transfers (wq0a on sync, wq0b+xq0
            # on scalar); both halves accumulate in one psum bank
            with tc.tile_wait_until(1):
                for qt in range(QT):
                    ps = psum.tile([128, NB], F32, tag="mm")
                    nh = NB // 2
                    for h in range(2):
                        w_sb = wq0a_sb if h == 0 else wq0b_sb
                        for t in range(kd_tiles // 2):
                            nc.tensor.matmul(
                                ps[:, h * nh:(h + 1) * nh],
                                lhsT=x2(t, qt),
                                rhs=w_sb[:, 2 * t:2 * t + 2, :],
                                start=(t == 0),
                                stop=(t == kd_tiles // 2 - 1),
                                perf_mode=DR,
                            )
                    nc.scalar.copy(q_sb[:, qt, 0:NB], ps)
            with tc.tile_wait_until(2):
                for qt in range(QT):
                    mm_dr(q_sb, qt, 1, lambda t, qt=qt: x2(t, qt), wq1_sb)
            with tc.tile_wait_until(3):
                for qt in range(QT):
                    mm_dr(kd_sb, qt, 0,
                          lambda t, qt=qt: cdiff8_2(t, qt), wk0_sb)
                    mm_dr(kd_sb, qt, 1,
                          lambda t, qt=qt: cdiff8_2(t, qt), wk1_sb)
                    if qt >= 1:
                        score(qt - 1)
            with tc.tile_wait_until(4):
                for qt in range(QT):
                    mm_dr(v_sb, qt, 0,
                          lambda t, qt=qt: cdiff8_2(t, qt), wv0_sb)
                    mm_dr(v_sb, qt, 1,
                          lambda t, qt=qt: cdiff8_2(t, qt), wv1_sb)
                    if qt >= 1:
                        avmul(qt - 1)
                    if qt == 1:
                        score(QT - 1)
                avmul(QT - 1)

            avT_sb = acts.tile([128, DT, QL], FB)

            def transposes(qt):
                for db in range(DT):
                    tp = psum_tr.tile([128, 128], FB, tag="tr")
                    nc.tensor.transpose(tp,
                                        av_sb[:, qt, db * 128:(db + 1) * 128],
                                        ident)
                    nc.scalar.copy(avT_sb[:, db, qt * 128:(qt + 1) * 128], tp)

            def o_group(qt):
                pss = [psum.tile([128, NB], F32, tag="mm", name=f"psg{jb}")
                       for jb in range(JT)]
                for jb in range(JT):
                    for kd in range(kd_tiles):
                        nc.tensor.matmul(
                            pss[jb],
                            lhsT=codd_slice(kd, qt * 128),
                            rhs=wvo_sb[:, kd, jb * NB:(jb + 1) * NB],
                            start=(kd == 0),
                            stop=False,
                        )
                    for kd in range(DT):
                        nc.tensor.matmul(
                            pss[jb],
                            lhsT=avT_sb[:, kd, qt * 128:(qt + 1) * 128],
                            rhs=wo_sb[:, kd, jb * NB:(jb + 1) * NB],
                            start=False,
                            stop=(kd == DT - 1),
                        )
                # copies split ACT/DVE into 256-col halves, DMA'd on both
                # rings: halves the post-last-matmul tail
                # h0: ACT copy then ACT-ring DMA (self-sequenced on ACT);
                # h1: DVE copy then sync-ring DMA — two independent
                # copy->issue->transfer chains per group
                for jb in range(JT):
                    o_t = outs.tile([128, NB], F32, tag="o")
                    nh = NB // 2
                    nc.scalar.copy(o_t[:, 0:nh], pss[jb][:, 0:nh])
                    nc.scalar.dma_start(
                        out=out[:, qt, jb * NB:jb * NB + nh], in_=o_t[:, 0:nh])
                    nc.vector.tensor_copy(o_t[:, nh:], pss[jb][:, nh:])
                    nc.sync.dma_start(
                        out=out[:, qt, jb * NB + nh:(jb + 1) * NB],
                        in_=o_t[:, nh:])

            with tc.tile_wait_until(5):
                transposes(0)
                transposes(1)
                o_group(0)
                transposes(2)
                o_group(1)
                transposes(3)
                o_group(2)
                o_group(3)

    nc.finalize()
    return nc


_GRAPH_CACHE = {}


def _get_graph(kd_tiles: int, with_bo: bool, fp8: bool = False):
    key = (kd_tiles, with_bo, fp8)
    if key not in _GRAPH_CACHE:
        _GRAPH_CACHE[key] = _build(kd_tiles, with_bo, fp8)
    return _GRAPH_CACHE[key]


def _pmajor(a, kd_tiles):
    """[kd_tiles*128, n] -> [128, kd_tiles, n] partition-major, contiguous."""
    n = a.shape[1]
    return np.ascontiguousarray(
        a.reshape(kd_tiles, 128, n).transpose(1, 0, 2))


def _make_in_maps(x, c, Wq, bq, Wk, bk, Wv, bv, Wo, bo):
    x = np.asarray(x, np.float32)
    c = np.asarray(c, np.float32)
    has_bias = any(np.any(np.asarray(b)) for b in (bq, bk, bv))
    with_bo = bool(np.any(np.asarray(bo)))
    fp8 = not has_bias and not with_bo
    kd_tiles = DT + (1 if has_bias else 0)
    KD = kd_tiles * 128

    if fp8:
        wqT8 = _pmajor(np.ascontiguousarray(
            np.asarray(Wq, np.float32).T * WSCALE).astype(E4M3), DT)
        wkT8 = _pmajor(np.ascontiguousarray(
            np.asarray(Wk, np.float32).T * WSCALE).astype(E4M3), DT)
        wvT8 = _pmajor(np.ascontiguousarray(
            np.asarray(Wv, np.float32).T * WSCALE).astype(E4M3), DT)
        Wo32 = np.asarray(Wo, np.float32)
        wvo_h = _pmajor(np.ascontiguousarray(
            (Wo32 @ np.asarray(Wv, np.float32)).T).astype(BF), DT)
        # v_sb carries 64 * (c_diff @ (Wv/2)^T); fold the 1/64 into wo
        wo_h = _pmajor(np.ascontiguousarray(
            Wo32.T / (2 * WSCALE)).astype(BF), DT)
        nh = NB // 2
        shared = {
            "wq0a": np.ascontiguousarray(wqT8[:, :, 0:nh]),
            "wq0b": np.ascontiguousarray(wqT8[:, :, nh:NB]),
            "wq1": np.ascontiguousarray(wqT8[:, :, NB:]),
            "wk0": np.ascontiguousarray(wkT8[:, :, 0:NB]),
            "wk1": np.ascontiguousarray(wkT8[:, :, NB:]),
            "wv0": np.ascontiguousarray(wvT8[:, :, 0:NB]),
            "wv1": np.ascontiguousarray(wvT8[:, :, NB:]),
            "wvo": wvo_h,
            "wo": wo_h,
        }
        in_maps = []
        for core in range(N_CORES):
            b = core // (N_CORES // B)
            q0 = (core % (N_CORES // B)) * QL
            k0 = 2 * q0
            xs = x[b, q0:q0 + QL]
            cs = c[b, k0:k0 + KL]
            c_mean = 0.5 * (cs[0::2] + cs[1::2])
            c_diff = cs[0::2] - cs[1::2]
            xT8 = _pmajor(np.ascontiguousarray(xs.T).astype(E4M3), DT)
            m = dict(shared)
            m.update({
                "xq0": np.ascontiguousarray(xT8[:, :, 0:128]),
                "xq1": np.ascontiguousarray(xT8[:, :, 128:256]),
                "xq2": np.ascontiguousarray(xT8[:, :, 256:384]),
                "xq3": np.ascontiguousarray(xT8[:, :, 384:512]),
                "cdf8": _pmajor(
                    np.ascontiguousarray(c_diff.T).astype(E4M3), DT),
                "cod": _pmajor(
                    np.ascontiguousarray(c_mean.T).astype(BF), DT),
            })
            in_maps.append(m)
        return in_maps, kd_tiles, with_bo, True

    def aug_w(W, b):
        wT = np.asarray(W, np.float32).T          # [D, D] feature-major
        if has_bias:
            pad = np.zeros((KD - D, D), np.float32)
            pad[0, :] = np.asarray(b, np.float32)
            wT = np.concatenate([wT, pad], axis=0)
        return _pmajor(wT.astype(BF), kd_tiles)

    wq_h = aug_w(Wq, bq)
    wk_h = aug_w(Wk, bk)
    wv_h = aug_w(Wv, bv)
    Wo32 = np.asarray(Wo, np.float32)
    wvo_h = aug_w(Wo32 @ np.asarray(Wv, np.float32),
                  Wo32 @ np.asarray(bv, np.float32))
    woT = np.ascontiguousarray(Wo32.T)
    if has_bias:
        woT = np.concatenate([woT, np.zeros((KD - D, D), np.float32)], axis=0)
    wo_h = _pmajor(woT.astype(BF), kd_tiles)

    def aug_act(aT, pad_val=1.0):
        if has_bias:
            pad = np.zeros((KD - D, aT.shape[1]), np.float32)
            pad[0, :] = pad_val
            aT = np.concatenate([aT, pad], axis=0)
        return _pmajor(aT.astype(BF), kd_tiles)

    in_maps = []
    for core in range(N_CORES):
        b = core // (N_CORES // B)
        q0 = (core % (N_CORES // B)) * QL
        k0 = 2 * q0
        xs = x[b, q0:q0 + QL]                      # [QL, D]
        cs = c[b, k0:k0 + KL]                      # [KL, D]
        c_odd = cs[1::2]                           # [QL, D]
        c_diff = cs[0::2] - cs[1::2]               # [QL, D], fp32 exact
        xT_h = aug_act(np.ascontiguousarray(xs.T))        # [128, kd, QL]
        codT_h = aug_act(np.ascontiguousarray(c_odd.T))   # bias row active
        cdifT_h = aug_act(np.ascontiguousarray(c_diff.T), pad_val=0.0)
        X0Q = 3 * 128
        m = {
            "xw0": np.ascontiguousarray(
                np.concatenate([xT_h[:, :, 0:X0Q], wq_h[:, :, 0:NB]], axis=2)),
            "xw1": np.ascontiguousarray(
                np.concatenate([xT_h[:, :, X0Q:], wq_h[:, :, NB:]], axis=2)),
            "ck": np.ascontiguousarray(np.concatenate([cdifT_h, wk_h], axis=2)),
            "cv": np.ascontiguousarray(np.concatenate([codT_h, wv_h], axis=2)),
            "woo": np.ascontiguousarray(np.concatenate([wo_h, wvo_h], axis=2)),
        }
        if with_bo:
            m["bo"] = np.asarray(bo, np.float32).reshape(1, D)
        in_maps.append(m)
    return in_maps, kd_tiles, with_bo, False


def _gather(results):
    out = np.empty((B, SQ, D), np.float32)
    for core in range(N_CORES):
        b = core // (N_CORES // B)
        q0 = (core % (N_CORES // B)) * QL
        arr = results[core]["out"]
        out[b, q0:q0 + QL] = arr.transpose(1, 0, 2).reshape(QL, D)
    return out


def kernel(**inputs) -> np.ndarray:
    in_maps, kd_tiles, with_bo, fp8 = _make_in_maps(**inputs)
    nc = _get_graph(kd_tiles, with_bo, fp8)
    res = run_bass_kernel_spmd(nc, in_maps, core_ids=list(range(N_CORES)))
    return _gather(res.results)


def run_traced(**inputs):
    """Like kernel() but with neuron-profile tracing; returns (out, results)."""
    in_maps, kd_tiles, with_bo, fp8 = _make_in_maps(**inputs)
    nc = _get_graph(kd_tiles, with_bo, fp8)
    res = run_bass_kernel_spmd(nc, in_maps, core_ids=list(range(N_CORES)),
                               trace=True)
    return _gather(res.results), res


# revision 21
# speedup vs baseline: 1.2194x; 1.2194x over previous
"""Sparse (block-local) attention for B=2, Sq=2048, Sk=4096, D=1024, H=16.

Each query i attends to exactly keys {2i, 2i+1} (Sk/Sq == 2, no remainder),
so softmax is over 2 scores -> p1 = sigmoid((s1-s2)*scale), p2 = 1-p1.

Distribution: sequence-parallel over (batch, query-block). 8 cores, each takes
512 contiguous queries of one batch plus the matching 1024 contiguous keys.
No collectives needed; outputs are concatenated on the host.

Algebraic cuts: with exactly 2 keys per query, softmax only needs the score
DIFFERENCE, and k_even - k_odd = (c_even - c_odd) @ Wk^T is linear, so the K
projection runs on c_diff = c_even - c_odd (512 rows, not 1024). Likewise
att = v_odd + p1 * (v_even - v_odd) reuses c_diff for V, and the v_odd term
folds through the output projection with a host-precomputed weight product
Wvo = Wo @ Wv:
  out = c_odd @ Wvo^T + (p1 * Vd) @ Wo^T,  Vd = c_diff @ Wv^T

Per-core device kernel (fp32 PSUM accumulation everywhere):
  Q  = x_s @ Wq^T     fp8 e4m3 DoubleRow (2 contraction rows/cycle, 2x rate)
  Kd = c_diff @ Wk^T  fp8 e4m3 DoubleRow
  s-diff row-wise dots on DVE per 64-dim head; p1 on ACT (sigmoid)
  Vd = c_diff @ Wv^T  bf16 (feeds the output directly -> fp8 too lossy)
  av = p1 * Vd on DVE; av^T via PE transposes
  O  = c_odd @ Wvo^T + av^T-matmul @ Wo^T   bf16

fp8 error budget (verified against an exact numpy replica): Wq/Wk ship as
e4m3 pre-scaled by 32 (folded into the sigmoid scale), x and c_diff as plain
e4m3. Only the sigmoid INPUT sees the quantization noise, so the output rel
err is ~1.6e-2 vs the 2e-2 gate; bf16 everywhere the error hits the output
linearly.

DMA: fine-grained whole tensors in phase need-order split across both hwdge
rings, no completion chains (per-ring FIFO keeps order); phase order is
pinned via tile_wait_until. Output copies split ACT/DVE into 256-col halves
DMA'd on both rings to shorten the tail.
"""

import sys

for _p in ("/opt/trn_rl_repo",):
    if _p not in sys.path:
        sys.path.append(_p)

import numpy as np
import ml_dtypes

import concourse.bass as bass
import concourse.mybir as mybir
import concourse.tile as tile
from concourse import bacc
from concourse.bass_utils import run_bass_kernel_spmd
from concourse.masks import make_identity
from concourse.tile_rust import add_dep_helper

B, SQ, SK, D, H, HD = 2, 2048, 4096, 1024, 16, 64
N_CORES = 8
QL = B * SQ // N_CORES       # 512 queries per core
KL = 2 * QL                  # 1024 keys per core
QT = QL // 128               # 4 query tiles
NB = 512                     # psum bank width (fp32)
JT = D // NB                 # 2 output-column blocks per projection
DT = D // 128                # 8 feature tiles
SCALE = 1.0 / float(np.sqrt(HD))

FB = mybir.dt.bfloat16
F32 = mybir.dt.float32
F8 = mybir.dt.float8e4
BF = ml_dtypes.bfloat16
E4M3 = ml_dtypes.float8_e4m3fn
WSCALE = 32.0


def _build(kd_tiles: int, with_bo: bool, fp8: bool = False):
    """Build + finalize the per-core Bacc graph (SPMD: same graph on 8 cores).

    fp8=True is the fast path for the bias-free case; the general
    (with-bias) path keeps everything bf16 with bias rows augmented into
    the contraction dim.
    """
    if fp8:
        return _build_fp8()
    nc = bacc.Bacc("TRN2", target_bir_lowering=False)

    # All activation/weight inputs are host-arranged partition-major:
    # tensor[p, t, n] = logical[t*128 + p, n], so DMA descriptors are
    # per-partition contiguous. Inputs are merged by NEED ORDER and the
    # DMA chain is gated so each phase gets full HBM bandwidth.
    X0Q = 3 * 128               # x columns (queries) in xw0
    xw0 = nc.dram_tensor("xw0", [128, kd_tiles, X0Q + NB], FB,
                         kind="ExternalInput")
    xw1 = nc.dram_tensor("xw1", [128, kd_tiles, (QL - X0Q) + (D - NB)], FB,
                         kind="ExternalInput")
    ck = nc.dram_tensor("ck", [128, kd_tiles, QL + D], FB, kind="ExternalInput")
    cv = nc.dram_tensor("cv", [128, kd_tiles, QL + D], FB, kind="ExternalInput")
    woo = nc.dram_tensor("woo", [128, kd_tiles, 2 * D], FB,
                         kind="ExternalInput")
    bo = None
    if with_bo:
        bo = nc.dram_tensor("bo", [1, D], F32, kind="ExternalInput")
    out = nc.dram_tensor("out", [128, QT, D], F32, kind="ExternalOutput")

    with tile.TileContext(nc) as tc:
        with (
            tc.tile_pool(name="ins", bufs=1) as ins,
            tc.tile_pool(name="acts", bufs=1) as acts,
            tc.tile_pool(name="att", bufs=4) as att,
            tc.tile_pool(name="outs", bufs=4) as outs,
            tc.tile_pool(name="psum", bufs=6, space="PSUM") as psum,
            tc.tile_pool(name="psum_tr", bufs=2, space="PSUM") as psum_tr,
        ):
            # ---- inputs to SBUF (need-order chained DMAs) ------------------
            xw0_sb = ins.tile([128, kd_tiles, X0Q + NB], FB)
            xw1_sb = ins.tile([128, kd_tiles, (QL - X0Q) + (D - NB)], FB)
            ck_sb = ins.tile([128, kd_tiles, QL + D], FB)
            cv_sb = ins.tile([128, kd_tiles, QL + D], FB)
            woo_sb = ins.tile([128, kd_tiles, 2 * D], FB)
            ident = ins.tile([128, 128], FB)

            h0 = (X0Q + NB) // 2
            d0a = nc.sync.dma_start(out=xw0_sb[:, :, 0:h0], in_=xw0[:, :, 0:h0])
            d0b = nc.scalar.dma_start(out=xw0_sb[:, :, h0:], in_=xw0[:, :, h0:])
            d1 = nc.sync.dma_start(out=xw1_sb, in_=xw1[:])
            d2 = nc.sync.dma_start(out=ck_sb, in_=ck[:])
            d3 = nc.sync.dma_start(out=cv_sb, in_=cv[:])
            d4 = nc.sync.dma_start(out=woo_sb, in_=woo[:])
            for d0x in (d0a, d0b):
                add_dep_helper(d1.ins, d0x.ins, sync=True)
                add_dep_helper(d2.ins, d0x.ins, sync=True)
            add_dep_helper(d3.ins, d1.ins, sync=True)
            add_dep_helper(d3.ins, d2.ins, sync=True)
            add_dep_helper(d4.ins, d3.ins, sync=True)
            bo_sb = None
            if with_bo:
                bo_sb = ins.tile([128, D], F32)
                d5 = nc.sync.dma_start(out=bo_sb,
                                       in_=bo[:].to_broadcast((128, D)))
                add_dep_helper(d5.ins, d3.ins, sync=True)
            make_identity(nc, ident)

            # PE warm-up: dummy matmuls during the DMA head keep HAM busy so
            # the real stream starts at full clock, at zero wall-clock cost.
            warm = ins.tile([128, 128], FB)
            nc.vector.memset(warm, 1.0)
            wps = psum_tr.tile([128, 128], F32, tag="tr")
            for _ in range(110):
                nc.tensor.matmul(wps, lhsT=warm, rhs=warm, start=True, stop=True)

            def x_slice(kd, col0):
                if col0 < X0Q:
                    return xw0_sb[:, kd, col0:col0 + 128]
                c = col0 - X0Q
                return xw1_sb[:, kd, c:c + 128]

            def wq_slice(kd, jb):
                if jb == 0:
                    return xw0_sb[:, kd, X0Q:X0Q + NB]
                c = (QL - X0Q) + (jb - 1) * NB
                return xw1_sb[:, kd, c:c + NB]

            def cdiff_slice(kd, col0):
                return ck_sb[:, kd, col0:col0 + 128]

            def wk_slice(kd, jb):
                return ck_sb[:, kd, QL + jb * NB:QL + (jb + 1) * NB]

            def codd_slice(kd, col0):
                return cv_sb[:, kd, col0:col0 + 128]

            def wv_slice(kd, jb):
                return cv_sb[:, kd, QL + jb * NB:QL + (jb + 1) * NB]

            # ---- projections (psum copies all on ACT) ----------------------
            q_sb = acts.tile([128, QT, D], FB)           # Q row-major
            kd_sb = acts.tile([128, QT, D], FB)          # Kd = c_diff @ Wk^T
            v_sb = acts.tile([128, QT, D], FB)           # Vd = c_diff @ Wv^T

            def mm_one(dst_tile, dst_idx, jb, lhs_fn, rhs_fn, nkd=kd_tiles):
                ps = psum.tile([128, NB], F32, tag="mm")
                for kd in range(nkd):
                    nc.tensor.matmul(
                        ps,
                        lhsT=lhs_fn(kd),
                        rhs=rhs_fn(kd, jb),
                        start=(kd == 0),
                        stop=(kd == nkd - 1),
                    )
                nc.scalar.copy(dst_tile[:, dst_idx, jb * NB:(jb + 1) * NB], ps)

            def mm_group(dst_tile, dst_idx, lhs_fn, rhs_fn):
                for jb in range(JT):
                    mm_one(dst_tile, dst_idx, jb, lhs_fn, rhs_fn)

            av_sb = acts.tile([128, QT, D], FB)

            def attention(qt):
                qv = q_sb[:, qt, :]
                kdv = kd_sb[:, qt, :]
                pe = att.tile([128, H, HD], FB, tag="prod")
                nc.vector.tensor_mul(pe.rearrange("p h e -> p (h e)"), qv, kdv)
                ds = att.tile([128, H], F32, tag="s")
                nc.vector.reduce_sum(out=ds, in_=pe, axis=mybir.AxisListType.X)
                p1 = att.tile([128, H], F32, tag="s")
                nc.scalar.activation(p1, ds, mybir.ActivationFunctionType.Sigmoid,
                                     scale=SCALE)
                vd = v_sb[:, qt, :].rearrange("p (h e) -> p h e", h=H)
                nc.vector.tensor_mul(
                    av_sb[:, qt, :].rearrange("p (h e) -> p h e", h=H),
                    vd, p1.to_broadcast((128, H, HD)))

            for jb in range(JT):
                for qt in range(QT):
                    mm_one(q_sb, qt, jb,
                           lambda kd, qt=qt: x_slice(kd, qt * 128), wq_slice)
            for qt in range(QT):
                mm_group(kd_sb, qt,
                         lambda kd, qt=qt: cdiff_slice(kd, qt * 128), wk_slice)
            for qt in range(QT):
                mm_group(v_sb, qt,
                         lambda kd, qt=qt: cdiff_slice(kd, qt * 128), wv_slice)
                if qt >= 1:
                    attention(qt - 1)
            attention(QT - 1)

            # ---- transpose att -> attT (copies on ACT), O groups interleaved
            avT_sb = acts.tile([128, DT, QL], FB)        # att^T feature-major

            def transposes(qt):
                for db in range(DT):
                    tp = psum_tr.tile([128, 128], FB, tag="tr")
                    nc.tensor.transpose(tp, av_sb[:, qt, db * 128:(db + 1) * 128],
                                        ident)
                    nc.scalar.copy(avT_sb[:, db, qt * 128:(qt + 1) * 128], tp)

            def o_group(qt):
                pss = [psum.tile([128, NB], F32, tag="mm", name=f"psg{jb}") for jb in range(JT)]
                for jb in range(JT):
                    for kd in range(kd_tiles):
                        nc.tensor.matmul(
                            pss[jb],
                            lhsT=codd_slice(kd, qt * 128),
                            rhs=woo_sb[:, kd, D + jb * NB:D + (jb + 1) * NB],
                            start=(kd == 0),
                            stop=False,
                        )
                    for kd in range(DT):
                        nc.tensor.matmul(
                            pss[jb],
                            lhsT=avT_sb[:, kd, qt * 128:(qt + 1) * 128],
                            rhs=woo_sb[:, kd, jb * NB:(jb + 1) * NB],
                            start=False,
                            stop=(kd == DT - 1),
                        )
                for jb in range(JT):
                    o_t = outs.tile([128, NB], F32, tag="o")
                    if with_bo:
                        nc.vector.tensor_add(o_t, pss[jb],
                                             bo_sb[:, jb * NB:(jb + 1) * NB])
                    elif jb % 2 == 0:
                        nc.scalar.copy(o_t, pss[jb])
                    else:
                        nc.vector.tensor_copy(o_t, pss[jb])
                    nc.sync.dma_start(out=out[:, qt, jb * NB:(jb + 1) * NB],
                                      in_=o_t)

            transposes(0)
            transposes(1)
            o_group(0)
            transposes(2)
            o_group(1)
            transposes(3)
            o_group(2)
            o_group(3)

    nc.finalize()
    return nc


def _build_fp8():
    """Bias-free fast path: fp8 DoubleRow Q/Kd, bf16 Vd/O (centered form).

    Centered attention: att = v_mean + (p1 - 1/2) * vd  with
    v_mean = (v_even + v_odd)/2; the coefficient t/2 = tanh(z/2)/2 has
    RMS ~0.15 instead of ~0.5, attenuating Vd-path rounding. Scales:
    Wv ships as bf16(32*Wv) so v_sb = 64*(c_diff @ (Wv/2)^T); the 1/64
    is folded into wo = Wo/64.

    Vd deliberately stays bf16: running Q+Kd+Vd all as fp8 DoubleRow on
    8 cores trips a package-level DVFS limit (whole-run PE cadence
    259ns vs 216ns, reproducible) that costs more than DoubleRow saves.
    The same DR volume on ONE core runs at full clock, so the limit is
    package power, not the NEFF mix.
    """
    nc = bacc.Bacc("TRN2", target_bir_lowering=False)
    kd_tiles = DT
    DR = mybir.MatmulPerfMode.DoubleRow

    xq0 = nc.dram_tensor("xq0", [128, kd_tiles, 128], F8, kind="ExternalInput")
    xq1 = nc.dram_tensor("xq1", [128, kd_tiles, 128], F8, kind="ExternalInput")
    xq2 = nc.dram_tensor("xq2", [128, kd_tiles, 128], F8, kind="ExternalInput")
    xq3 = nc.dram_tensor("xq3", [128, kd_tiles, 128], F8, kind="ExternalInput")
    wq0a = nc.dram_tensor("wq0a", [128, kd_tiles, NB // 2], F8,
                          kind="ExternalInput")
    wq0b = nc.dram_tensor("wq0b", [128, kd_tiles, NB // 2], F8,
                          kind="ExternalInput")
    wq1 = nc.dram_tensor("wq1", [128, kd_tiles, NB], F8, kind="ExternalInput")
    cdf8 = nc.dram_tensor("cdf8", [128, kd_tiles, QL], F8,
                          kind="ExternalInput")
    wk0 = nc.dram_tensor("wk0", [128, kd_tiles, NB], F8, kind="ExternalInput")
    wk1 = nc.dram_tensor("wk1", [128, kd_tiles, NB], F8, kind="ExternalInput")
    # Vd stays bf16: a higher fp8-DoubleRow mix (Q+Kd+Vd) trips the device
    # into a 2.0 GHz PE clock for the WHOLE run (observed 259ns vs 216ns
    # per-matmul cadence, reproducible), costing more than DoubleRow saves.
    cdf = nc.dram_tensor("cdf", [128, kd_tiles, QL], FB, kind="ExternalInput")
    wv0 = nc.dram_tensor("wv0", [128, kd_tiles, NB], FB, kind="ExternalInput")
    wv1 = nc.dram_tensor("wv1", [128, kd_tiles, NB], FB, kind="ExternalInput")
    cod = nc.dram_tensor("cod", [128, kd_tiles, QL], FB, kind="ExternalInput")
    wvo = nc.dram_tensor("wvo", [128, kd_tiles, D], FB, kind="ExternalInput")
    wo = nc.dram_tensor("wo", [128, kd_tiles, D], FB, kind="ExternalInput")
    out = nc.dram_tensor("out", [128, QT, D], F32, kind="ExternalOutput")

    with tile.TileContext(nc) as tc:
        with (
            tc.tile_pool(name="ins", bufs=1) as ins,
            tc.tile_pool(name="acts", bufs=1) as acts,
            tc.tile_pool(name="att", bufs=4) as att,
            tc.tile_pool(name="outs", bufs=8) as outs,
            tc.tile_pool(name="psum", bufs=5, space="PSUM") as psum,
            tc.tile_pool(name="psum_tr", bufs=2, space="PSUM") as psum_tr,
            tc.tile_pool(name="psum_w", bufs=1, space="PSUM") as psum_w,
        ):
            xq_sb = [ins.tile([128, kd_tiles, 128], F8, name=f"xq{i}")
                     for i in range(QT)]
            wq0a_sb = ins.tile([128, kd_tiles, NB // 2], F8)
            wq0b_sb = ins.tile([128, kd_tiles, NB // 2], F8)
            wq1_sb = ins.tile([128, kd_tiles, NB], F8)
            cdf8_sb = ins.tile([128, kd_tiles, QL], F8)
            wk0_sb = ins.tile([128, kd_tiles, NB], F8)
            wk1_sb = ins.tile([128, kd_tiles, NB], F8)
            cdf_sb = ins.tile([128, kd_tiles, QL], FB)
            wv0_sb = ins.tile([128, kd_tiles, NB], FB)
            wv1_sb = ins.tile([128, kd_tiles, NB], FB)
            cod_sb = ins.tile([128, kd_tiles, QL], FB)
            wvo_sb = ins.tile([128, kd_tiles, D], FB)
            wo_sb = ins.tile([128, kd_tiles, D], FB)
            ident = ins.tile([128, 128], FB)

            # unchained: per-ring FIFO keeps need-order. Q-critical data is
            # split across both rings' crawl windows; each later phase's
            # tensors are balanced so both rings deliver it about when the
            # PE stream reaches it.
            nc.sync.dma_start(out=wq0a_sb, in_=wq0a[:])
            nc.sync.dma_start(out=xq_sb[1], in_=xq1[:])
            nc.sync.dma_start(out=xq_sb[2], in_=xq2[:])
            nc.sync.dma_start(out=xq_sb[3], in_=xq3[:])
            nc.sync.dma_start(out=wk0_sb, in_=wk0[:])
            nc.sync.dma_start(out=wk1_sb, in_=wk1[:])
            nc.sync.dma_start(out=cdf_sb, in_=cdf[:])
            nc.sync.dma_start(out=wv1_sb, in_=wv1[:])
            nc.sync.dma_start(out=wo_sb, in_=wo[:])
            nc.scalar.dma_start(out=xq_sb[0], in_=xq0[:])
            nc.scalar.dma_start(out=wq0b_sb, in_=wq0b[:])
            nc.scalar.dma_start(out=wq1_sb, in_=wq1[:])
            nc.scalar.dma_start(out=cdf8_sb, in_=cdf8[:])
            nc.scalar.dma_start(out=wv0_sb, in_=wv0[:])
            nc.scalar.dma_start(out=cod_sb, in_=cod[:])
            nc.scalar.dma_start(out=wvo_sb, in_=wvo[:])
            make_identity(nc, ident)

            # PE warm-up holds the p-state ramp until the first Q data lands
            warm = ins.tile([128, 128], FB)
            nc.vector.memset(warm, 1.0)
            wps = psum_w.tile([128, 128], F32, tag="warm")
            for _ in range(44):
                nc.tensor.matmul(wps, lhsT=warm, rhs=warm, start=True,
                                 stop=True)

            def x2(t, qt):
                # fp8 DoubleRow lhsT: contraction pair (2t, 2t+1), 128 q cols
                return xq_sb[qt][:, 2 * t:2 * t + 2, :]

            def cdiff8_2(t, qt):
                return cdf8_sb[:, 2 * t:2 * t + 2, qt * 128:(qt + 1) * 128]

            def codd_slice(kd, col0):
                return cod_sb[:, kd, col0:col0 + 128]

            q_sb = acts.tile([128, QT, D], FB)
            kd_sb = acts.tile([128, QT, D], FB)
            v_sb = acts.tile([128, QT, D], FB)

            def mm_dr(dst_tile, dst_idx, jb, lhs_fn, rhs_sb):
                # 4 DoubleRow matmuls, 256-contraction each
                ps = psum.tile([128, NB], F32, tag="mm")
                for t in range(kd_tiles // 2):
                    nc.tensor.matmul(
                        ps,
                        lhsT=lhs_fn(t),
                        rhs=rhs_sb[:, 2 * t:2 * t + 2, :],
                        start=(t == 0),
                        stop=(t == kd_tiles // 2 - 1),
                        perf_mode=DR,
                    )
                nc.scalar.copy(dst_tile[:, dst_idx, jb * NB:(jb + 1) * NB], ps)

            av_sb = acts.tile([128, QT, D], FB)

            # t = tanh(z/2) = 2*(p1 - 1/2); av = t * (64 * c_diff@(Wv/2)^T),
            # the 1/64 folded into wo host-side
            t_tiles = [None] * QT

            def score(qt):
                qv = q_sb[:, qt, :]
                kdv = kd_sb[:, qt, :]
                pe = att.tile([128, H, HD], FB, tag="prod")
                nc.vector.tensor_mul(pe.rearrange("p h e -> p (h e)"), qv, kdv)
                ds = att.tile([128, H], F32, tag="s")
                nc.vector.reduce_sum(out=ds, in_=pe, axis=mybir.AxisListType.X)
                t = att.tile([128, H], F32, tag="p1")
                nc.scalar.activation(t, ds,
                                     mybir.ActivationFunctionType.Tanh,
                                     scale=SCALE / (2 * WSCALE * WSCALE))
                t_tiles[qt] = t

            def avmul(qt):
                vd = v_sb[:, qt, :].rearrange("p (h e) -> p h e", h=H)
                nc.vector.tensor_mul(
                    av_sb[:, qt, :].rearrange("p (h e) -> p h e", h=H),
                    vd, t_tiles[qt].to_broadcast((128, H, HD)))

            # Q jb0 in 256-col halves so the first groups gate on the
            # smallest possible front transfers (wq0a on sync, wq0b+xq0
            # on scalar); both halves accumulate in one psum bank
            with tc.tile_wait_until(1):
                for qt in range(QT):
                    ps = psum.tile([128, NB], F32, tag="mm")
                    nh = NB // 2
                    for h in range(2):
                        w_sb = wq0a_sb if h == 0 else wq0b_sb
                        for t in range(kd_tiles // 2):
                            nc.tensor.matmul(
                                ps[:, h * nh:(h + 1) * nh],
                                lhsT=x2(t, qt),
                                rhs=w_sb[:, 2 * t:2 * t + 2, :],
                                start=(t == 0),
                                stop=(t == kd_tiles // 2 - 1),
                                perf_mode=DR,
                            )
                    nc.scalar.copy(q_sb[:, qt, 0:NB], ps)
            with tc.tile_wait_until(2):
                for qt in range(QT):
                    mm_dr(q_sb, qt, 1, lambda t, qt=qt: x2(t, qt), wq1_sb)
            def mm_one(dst_tile, dst_idx, jb, lhs_fn, rhs_fn):
                # bf16 group: 8 single-row matmuls, 128-contraction each
                ps = psum.tile([128, NB], F32, tag="mm")
                for kd in range(kd_tiles):
                    nc.tensor.matmul(
                        ps,
                        lhsT=lhs_fn(kd),
                        rhs=rhs_fn(kd),
                        start=(kd == 0),
                        stop=(kd == kd_tiles - 1),
                    )
                nc.scalar.copy(dst_tile[:, dst_idx, jb * NB:(jb + 1) * NB], ps)

            def cdiff_slice(kd, col0):
                return cdf_sb[:, kd, col0:col0 + 128]

            avT_sb = acts.tile([128, DT, QL], FB)

            def transposes(qt):
                # 8 PE transposes into one psum bank, then ONE batched ACT
                # copy (per-op ACT overhead made the copies the transpose
                # stream's bottleneck when done 128 cols at a time)
                tp = psum_tr.tile([128, DT, 128], FB, tag="tr")
                for db in range(DT):
                    nc.tensor.transpose(tp[:, db, :],
                                        av_sb[:, qt, db * 128:(db + 1) * 128],
                                        ident)
                nc.scalar.copy(avT_sb[:, :, qt * 128:(qt + 1) * 128], tp)

            # jb-outer so each phase's first groups only gate on that
            # phase's first weight tensor
            with tc.tile_wait_until(3):
                for qt in range(QT):
                    mm_dr(kd_sb, qt, 0,
                          lambda t, qt=qt: cdiff8_2(t, qt), wk0_sb)
                for qt in range(QT):
                    mm_dr(kd_sb, qt, 1,
                          lambda t, qt=qt: cdiff8_2(t, qt), wk1_sb)
                    if qt >= 1:
                        score(qt - 1)
            with tc.tile_wait_until(4):
                score(QT - 1)
                for qt in range(QT):
                    mm_one(v_sb, qt, 0,
                           lambda kd, qt=qt: cdiff_slice(kd, qt * 128),
                           lambda kd: wv0_sb[:, kd, :])
                for qt in range(QT):
                    mm_one(v_sb, qt, 1,
                           lambda kd, qt=qt: cdiff_slice(kd, qt * 128),
                           lambda kd: wv1_sb[:, kd, :])
                    if qt >= 1:
                        avmul(qt - 1)
                    if qt >= 2:
                        transposes(qt - 2)
                avmul(QT - 1)

            o_pss = {}

            def o_codd(qt):
                # pure half: c_mean @ Wvo^T, no attention dependency
                pss = [psum.tile([128, NB], F32, tag="mm", name=f"psg{jb}")
                       for jb in range(JT)]
                o_pss[qt] = pss
                for jb in range(JT):
                    for kd in range(kd_tiles):
                        nc.tensor.matmul(
                            pss[jb],
                            lhsT=codd_slice(kd, qt * 128),
                            rhs=wvo_sb[:, kd, jb * NB:(jb + 1) * NB],
                            start=(kd == 0),
                            stop=False,
                        )

            def o_avt(qt):
                pss = o_pss.pop(qt)
                for jb in range(JT):
                    for kd in range(DT):
                        nc.tensor.matmul(
                            pss[jb],
                            lhsT=avT_sb[:, kd, qt * 128:(qt + 1) * 128],
                            rhs=wo_sb[:, kd, jb * NB:(jb + 1) * NB],
                            start=False,
                            stop=(kd == DT - 1),
                        )
                # copies split ACT/DVE into 256-col halves, DMA'd on both
                # rings: h0: ACT copy then ACT-ring DMA (self-sequenced on
                # ACT); h1: DVE copy then sync-ring DMA
                for jb in range(JT):
                    o_t = outs.tile([128, NB], F32, tag="o")
                    nh = NB // 2
                    nc.scalar.copy(o_t[:, 0:nh], pss[jb][:, 0:nh])
                    nc.scalar.dma_start(
                        out=out[:, qt, jb * NB:jb * NB + nh], in_=o_t[:, 0:nh])
                    nc.vector.tensor_copy(o_t[:, nh:], pss[jb][:, nh:])
                    nc.sync.dma_start(
                        out=out[:, qt, jb * NB + nh:(jb + 1) * NB],
                        in_=o_t[:, nh:])

            with tc.tile_wait_until(5):
                transposes(2)
                o_codd(0)
                transposes(3)
                o_avt(0)
                o_codd(1)
                o_avt(1)
                o_codd(2)
                o_avt(2)
                o_codd(3)
                o_avt(3)

    nc.finalize()
    return nc


_GRAPH_CACHE = {}


def _get_graph(kd_tiles: int, with_bo: bool, fp8: bool = False):
    key = (kd_tiles, with_bo, fp8)
    if key not in _GRAPH_CACHE:
        _GRAPH_CACHE[key] = _build(kd_tiles, with_bo, fp8)
    return _GRAPH_CACHE[key]


def _pmajor(a, kd_tiles):
    """[kd_tiles*128, n] -> [128, kd_tiles, n] partition-major, contiguous."""
    n = a.shape[1]
    return np.ascontiguousarray(
        a.reshape(kd_tiles, 128, n).transpose(1, 0, 2))


def _make_in_maps(x, c, Wq, bq, Wk, bk, Wv, bv, Wo, bo):
    x = np.asarray(x, np.float32)
    c = np.asarray(c, np.float32)
    has_bias = any(np.any(np.asarray(b)) for b in (bq, bk, bv))
    with_bo = bool(np.any(np.asarray(bo)))
    fp8 = not has_bias and not with_bo
    kd_tiles = DT + (1 if has_bias else 0)
    KD = kd_tiles * 128

    if fp8:
        wqT8 = _pmajor(np.ascontiguousarray(
            np.asarray(Wq, np.float32).T * WSCALE).astype(E4M3), DT)
        wkT8 = _pmajor(np.ascontiguousarray(
            np.asarray(Wk, np.float32).T * WSCALE).astype(E4M3), DT)
        wvT16 = _pmajor(np.ascontiguousarray(
            np.asarray(Wv, np.float32).T * WSCALE).astype(BF), DT)
        Wo32 = np.asarray(Wo, np.float32)
        wvo_h = _pmajor(np.ascontiguousarray(
            (Wo32 @ np.asarray(Wv, np.float32)).T).astype(BF), DT)
        # v_sb carries 64 * (c_diff @ (Wv/2)^T); fold the 1/64 into wo
        wo_h = _pmajor(np.ascontiguousarray(
            Wo32.T / (2 * WSCALE)).astype(BF), DT)
        nh = NB // 2
        shared = {
            "wq0a": np.ascontiguousarray(wqT8[:, :, 0:nh]),
            "wq0b": np.ascontiguousarray(wqT8[:, :, nh:NB]),
            "wq1": np.ascontiguousarray(wqT8[:, :, NB:]),
            "wk0": np.ascontiguousarray(wkT8[:, :, 0:NB]),
            "wk1": np.ascontiguousarray(wkT8[:, :, NB:]),
            "wv0": np.ascontiguousarray(wvT16[:, :, 0:NB]),
            "wv1": np.ascontiguousarray(wvT16[:, :, NB:]),
            "wvo": wvo_h,
            "wo": wo_h,
        }
        in_maps = []
        for core in range(N_CORES):
            b = core // (N_CORES // B)
            q0 = (core % (N_CORES // B)) * QL
            k0 = 2 * q0
            xs = x[b, q0:q0 + QL]
            cs = c[b, k0:k0 + KL]
            c_mean = 0.5 * (cs[0::2] + cs[1::2])
            c_diff = cs[0::2] - cs[1::2]
            xT8 = _pmajor(np.ascontiguousarray(xs.T).astype(E4M3), DT)
            m = dict(shared)
            m.update({
                "xq0": np.ascontiguousarray(xT8[:, :, 0:128]),
                "xq1": np.ascontiguousarray(xT8[:, :, 128:256]),
                "xq2": np.ascontiguousarray(xT8[:, :, 256:384]),
                "xq3": np.ascontiguousarray(xT8[:, :, 384:512]),
                "cdf8": _pmajor(
                    np.ascontiguousarray(c_diff.T).astype(E4M3), DT),
                "cdf": _pmajor(
                    np.ascontiguousarray(c_diff.T).astype(BF), DT),
                "cod": _pmajor(
                    np.ascontiguousarray(c_mean.T).astype(BF), DT),
            })
            in_maps.append(m)
        return in_maps, kd_tiles, with_bo, True

    def aug_w(W, b):
        wT = np.asarray(W, np.float32).T          # [D, D] feature-major
        if has_bias:
            pad = np.zeros((KD - D, D), np.float32)
            pad[0, :] = np.asarray(b, np.float32)
            wT = np.concatenate([wT, pad], axis=0)
        return _pmajor(wT.astype(BF), kd_tiles)

    wq_h = aug_w(Wq, bq)
    wk_h = aug_w(Wk, bk)
    wv_h = aug_w(Wv, bv)
    Wo32 = np.asarray(Wo, np.float32)
    wvo_h = aug_w(Wo32 @ np.asarray(Wv, np.float32),
                  Wo32 @ np.asarray(bv, np.float32))
    woT = np.ascontiguousarray(Wo32.T)
    if has_bias:
        woT = np.concatenate([woT, np.zeros((KD - D, D), np.float32)], axis=0)
    wo_h = _pmajor(woT.astype(BF), kd_tiles)

    def aug_act(aT, pad_val=1.0):
        if has_bias:
            pad = np.zeros((KD - D, aT.shape[1]), np.float32)
            pad[0, :] = pad_val
            aT = np.concatenate([aT, pad], axis=0)
        return _pmajor(aT.astype(BF), kd_tiles)

    in_maps = []
    for core in range(N_CORES):
        b = core // (N_CORES // B)
        q0 = (core % (N_CORES // B)) * QL
        k0 = 2 * q0
        xs = x[b, q0:q0 + QL]                      # [QL, D]
        cs = c[b, k0:k0 + KL]                      # [KL, D]
        c_odd = cs[1::2]                           # [QL, D]
        c_diff = cs[0::2] - cs[1::2]               # [QL, D], fp32 exact
        xT_h = aug_act(np.ascontiguousarray(xs.T))        # [128, kd, QL]
        codT_h = aug_act(np.ascontiguousarray(c_odd.T))   # bias row active
        cdifT_h = aug_act(np.ascontiguousarray(c_diff.T), pad_val=0.0)
        X0Q = 3 * 128
        m = {
            "xw0": np.ascontiguousarray(
                np.concatenate([xT_h[:, :, 0:X0Q], wq_h[:, :, 0:NB]], axis=2)),
            "xw1": np.ascontiguousarray(
                np.concatenate([xT_h[:, :, X0Q:], wq_h[:, :, NB:]], axis=2)),
            "ck": np.ascontiguousarray(np.concatenate([cdifT_h, wk_h], axis=2)),
            "cv": np.ascontiguousarray(np.concatenate([codT_h, wv_h], axis=2)),
            "woo": np.ascontiguousarray(np.concatenate([wo_h, wvo_h], axis=2)),
        }
        if with_bo:
            m["bo"] = np.asarray(bo, np.float32).reshape(1, D)
        in_maps.append(m)
    return in_maps, kd_tiles, with_bo, False


def _gather(results):
    out = np.empty((B, SQ, D), np.float32)
    for core in range(N_CORES):
        b = core // (N_CORES // B)
        q0 = (core % (N_CORES // B)) * QL
        arr = results[core]["out"]
        out[b, q0:q0 + QL] = arr.transpose(1, 0, 2).reshape(QL, D)
    return out


def kernel(**inputs) -> np.ndarray:
    in_maps, kd_tiles, with_bo, fp8 = _make_in_maps(**inputs)
    nc = _get_graph(kd_tiles, with_bo, fp8)
    res = run_bass_kernel_spmd(nc, in_maps, core_ids=list(range(N_CORES)))
    return _gather(res.results)


def run_traced(**inputs):
    """Like kernel() but with neuron-profile tracing; returns (out, results)."""
    in_maps, kd_tiles, with_bo, fp8 = _make_in_maps(**inputs)
    nc = _get_graph(kd_tiles, with_bo, fp8)
    res = run_bass_kernel_spmd(nc, in_maps, core_ids=list(range(N_CORES)),
                               trace=True)
    return _gather(res.results), res
